# revision 12
# baseline (speedup 1.0000x reference)
"""Embedding lookup kernel for Trainium2 (8 NeuronCores).

Problem: x [1, 8192] int token ids, weights [49408, 768] f32 table
         -> out [8192, 768] f32  (out[s] = weights[x[0, s]])

Strategy: data-parallel over the sequence. Each of the 8 cores gets the
full table (resident in its HBM) plus a 1024-token slice of ids, and
gathers its 1024 rows via SWDGE indirect DMA (one 3KB descriptor per
row) into SBUF, then stores each 128-row tile to its output slice.
Host concatenates the 8 slices. (A direct DRAM->DRAM gather would
halve the SDMA work, but that path is broken in hardware, as are
indirect DMAs touching fewer than 128 partitions.)

Raw Bass, no TileContext (its preamble + EVSEM drain tail cost ~16us
on a ~25us kernel). All buffers stay resident in SBUF so nothing
stalls on reuse. The index block loads in two DMAs so the first gather
waits on only 512B. Stores alternate between the two HWDGE engines
(sync/scalar), and the last tile's store is split in half across both
so the end-of-kernel completion chain is shorter.
"""

import numpy as np

import concourse.bass as bass
import concourse.mybir as mybir
from concourse.bass_utils import run_bass_kernel_spmd

SEQ = 8192
VOCAB = 49408
DIM = 768
NCORES = 8
P = 128

TOK = SEQ // NCORES  # tokens per core
TILES = TOK // P  # gather tiles of P tokens
BLOCKS = TILES

_cache = {}


def _build():
    """Per-core program: out[t*P+p, :] = weights[ids[p, t], :].

    ids arrives host-transposed: ids_a [P, 1] = block 0, ids_b [P, 7] =
    blocks 1-7 (column t = block t's P tokens), both contiguous.
    """
    import contextlib

    nc = bass.Bass()

    ids_a = nc.dram_tensor("ids_a", [P, 1], mybir.dt.int32, kind="ExternalInput")
    ids_b = nc.dram_tensor(
        "ids_b", [P, TILES - 1], mybir.dt.int32, kind="ExternalInput"
    )
    weights = nc.dram_tensor(
        "weights", [VOCAB, DIM], mybir.dt.float32, kind="ExternalInput"
    )
    out = nc.dram_tensor("out", [TOK, DIM], mybir.dt.float32, kind="ExternalOutput")
    out_t = out.rearrange("(t p) d -> t p d", p=P)  # [TILES, P, DIM]

    with contextlib.ExitStack() as ctx:
        idx_sb = ctx.enter_context(nc.sbuf_tensor([P, TILES], mybir.dt.int32))
        g_sb = ctx.enter_context(nc.sbuf_tensor([P, TILES * DIM], mybir.dt.float32))
        idx_sem_a = ctx.enter_context(nc.semaphore("idx_sem_a"))
        idx_sem_b = ctx.enter_context(nc.semaphore("idx_sem_b"))
        store_sem = ctx.enter_context(nc.semaphore("store_sem"))
        # one sem per gather: DMA sem updates may not cross other DMAs'
        # waited values (race detector), so don't share a counter
        gather_sems = [
            ctx.enter_context(nc.semaphore(f"gather_sem{t}")) for t in range(TILES)
        ]
        block = ctx.enter_context(nc.Block())

        last = TILES - 1

        def store(eng, t, lo=0, hi=P):
            eng.dma_start(
                out=out_t[t, lo:hi], in_=g_sb[lo:hi, t * DIM : (t + 1) * DIM]
            ).then_inc(store_sem, 16)

        # 8 gathers -> 9 stores (last tile split in half across engines)
        nstores = TILES + 1

        @block.sync
        def _(sync: bass.BassEngine):
            # ids for block 0 first: the first gather waits only on 512B
            sync.dma_start(out=idx_sb[:, 0:1], in_=ids_a[:]).then_inc(idx_sem_a, 16)
            sync.dma_start(out=idx_sb[:, 1:], in_=ids_b[:]).then_inc(idx_sem_b, 16)
            for t in range(0, TILES - 1, 2):
                sync.wait_ge(gather_sems[t], 16)
                store(sync, t)
            sync.wait_ge(gather_sems[last], 16)
            store(sync, last, 0, P // 2)
            sync.wait_ge(store_sem, 16 * nstores)

        @block.scalar
        def _(scalar: bass.BassEngine):
            for t in range(1, TILES - 1, 2):
                scalar.wait_ge(gather_sems[t], 16)
                store(scalar, t)
            scalar.wait_ge(gather_sems[last], 16)
            store(scalar, last, P // 2, P)

        @block.gpsimd
        def _(gpsimd: bass.BassEngine):
            gpsimd.wait_ge(idx_sem_a, 16)
            for t in range(TILES):
                if t == 1:
                    gpsimd.wait_ge(idx_sem_b, 16)
                gpsimd.indirect_dma_start(
                    out=g_sb[:, t * DIM : (t + 1) * DIM],
                    out_offset=None,
                    in_=weights[:],
                    in_offset=bass.IndirectOffsetOnAxis(
                        ap=idx_sb[:, t : t + 1], axis=0
                    ),
                ).then_inc(gather_sems[t], 16)

    return nc


def _get_nc():
    if "nc" not in _cache:
        _cache["nc"] = _build()
    return _cache["nc"]


def _run(x, weights, trace=False):
    ids = np.ascontiguousarray(np.asarray(x).reshape(-1).astype(np.int32))
    w = np.ascontiguousarray(np.asarray(weights, dtype=np.float32))
    assert ids.shape == (SEQ,) and w.shape == (VOCAB, DIM)

    nc = _get_nc()
    in_maps = []
    for c in range(NCORES):
        sl = ids[c * TOK : (c + 1) * TOK]
        in_maps.append(
            {
                # token t*P+p of this core's slice lands at idx_sb[p, t]
                "ids_a": np.ascontiguousarray(sl[:P].reshape(P, 1)),
                "ids_b": np.ascontiguousarray(sl[P:].reshape(TILES - 1, P).T),
                "weights": w,
            }
        )
    br = run_bass_kernel_spmd(nc, in_maps, list(range(NCORES)), trace=trace)
    out = np.concatenate([br.results[c]["out"] for c in range(NCORES)], axis=0)
    return out, br


def kernel(x, weights):
    out, _ = _run(x, weights)
    return out


# revision 21
# speedup vs baseline: 1.1288x; 1.1288x over previous
"""Embedding lookup kernel for Trainium2 (8 NeuronCores).

Problem: x [1, 8192] int token ids, weights [49408, 768] f32 table
         -> out [8192, 768] f32  (out[s] = weights[x[0, s]])

Strategy: data-parallel over the sequence. Each of the 8 cores gets the
full table (resident in its HBM) plus a 1024-token slice of ids, and
gathers its 1024 rows via SWDGE indirect DMA (one 3KB descriptor per
row) into SBUF, then stores each 128-row tile to its output slice.
Host concatenates the 8 slices. (A direct DRAM->DRAM gather would
halve the SDMA work, but that path is broken in hardware, as are
indirect DMAs touching fewer than 128 partitions.)

Raw Bass, no TileContext (its preamble + EVSEM drain tail cost ~16us
on a ~25us kernel). All buffers stay resident in SBUF so nothing
stalls on reuse. The ids load as one [128, 8] DMA, host-transposed so
gather t's offsets are column t (indirect offset APs must be [N, 1],
one index per partition — anything else dies on hardware). Stores
alternate between the two HWDGE engines (sync/scalar) with the last
tile's store split across both, and their semaphore is never waited
on: the exit-barrier Drain on each engine already waits for its
outstanding DMA data (verified in traces), so the kernel ends at drain
time instead of after another sem round-trip.
"""

import numpy as np

import concourse.bass as bass
import concourse.mybir as mybir
from concourse.bass_utils import run_bass_kernel_spmd

SEQ = 8192
VOCAB = 49408
DIM = 768
NCORES = 8
P = 128

TOK = SEQ // NCORES  # tokens per core
TILES = TOK // P  # gather tiles of P tokens

_cache = {}


def _build():
    """Per-core program: out[i, :] = weights[ids[i//64, i%64], :]."""
    import contextlib

    nc = bass.Bass()

    ids = nc.dram_tensor("ids", [P, TILES], mybir.dt.int32, kind="ExternalInput")
    weights = nc.dram_tensor(
        "weights", [VOCAB, DIM], mybir.dt.float32, kind="ExternalInput"
    )
    out = nc.dram_tensor("out", [TOK, DIM], mybir.dt.float32, kind="ExternalOutput")
    out_t = out.rearrange("(t p) d -> t p d", p=P)  # [TILES, P, DIM]

    with contextlib.ExitStack() as ctx:
        idx_sb = ctx.enter_context(nc.sbuf_tensor([P, TILES], mybir.dt.int32))
        g_sb = ctx.enter_context(nc.sbuf_tensor([P, TILES * DIM], mybir.dt.float32))
        idx_sem = ctx.enter_context(nc.semaphore("idx_sem"))
        store_sem = ctx.enter_context(nc.semaphore("store_sem"))
        # one sem per gather: DMA sem updates may not cross other DMAs'
        # waited values (race detector), so don't share a counter
        gather_sems = [
            ctx.enter_context(nc.semaphore(f"gather_sem{t}")) for t in range(TILES)
        ]
        block = ctx.enter_context(nc.Block())

        last = TILES - 1

        def store(eng, t, lo=0, hi=P):
            # store_sem is inc'd (every DMA must update a sem) but never
            # waited on: the engine's exit-barrier Drain already waits for
            # its outstanding DMA data, which is when the kernel may end
            eng.dma_start(
                out=out_t[t, lo:hi], in_=g_sb[lo:hi, t * DIM : (t + 1) * DIM]
            ).then_inc(store_sem, 16)

        @block.sync
        def _(sync: bass.BassEngine):
            sync.dma_start(out=idx_sb[:], in_=ids[:]).then_inc(idx_sem, 16)
            for t in range(0, TILES - 1, 2):
                sync.wait_ge(gather_sems[t], 16)
                store(sync, t)
            sync.wait_ge(gather_sems[last], 16)
            store(sync, last, 0, P // 2)

        @block.scalar
        def _(scalar: bass.BassEngine):
            for t in range(1, TILES - 1, 2):
                scalar.wait_ge(gather_sems[t], 16)
                store(scalar, t)
            scalar.wait_ge(gather_sems[last], 16)
            store(scalar, last, P // 2, P)

        @block.gpsimd
        def _(gpsimd: bass.BassEngine):
            gpsimd.wait_ge(idx_sem, 16)
            for t in range(TILES):
                gpsimd.indirect_dma_start(
                    out=g_sb[:, t * DIM : (t + 1) * DIM],
                    out_offset=None,
                    in_=weights[:],
                    in_offset=bass.IndirectOffsetOnAxis(
                        ap=idx_sb[:, t : t + 1], axis=0
                    ),
                ).then_inc(gather_sems[t], 16)

    return nc


def _get_nc():
    if "nc" not in _cache:
        _cache["nc"] = _build()
    return _cache["nc"]


def _run(x, weights, trace=False):
    ids = np.ascontiguousarray(np.asarray(x).reshape(-1).astype(np.int32))
    w = np.ascontiguousarray(np.asarray(weights, dtype=np.float32))
    assert ids.shape == (SEQ,) and w.shape == (VOCAB, DIM)

    nc = _get_nc()
    in_maps = [
        {
            # token t*P+p of this core's slice lands at idx_sb[p, t]
            "ids": np.ascontiguousarray(
                ids[c * TOK : (c + 1) * TOK].reshape(TILES, P).T
            ),
            "weights": w,
        }
        for c in range(NCORES)
    ]
    br = run_bass_kernel_spmd(nc, in_maps, list(range(NCORES)), trace=trace)
    out = np.concatenate([br.results[c]["out"] for c in range(NCORES)], axis=0)
    return out, br


def kernel(x, weights):
    out, _ = _run(x, weights)
    return out


# revision 23
# speedup vs baseline: 1.2372x; 1.0960x over previous
"""Embedding lookup kernel for Trainium2 (8 NeuronCores).

Problem: x [1, 8192] int token ids, weights [49408, 768] f32 table
         -> out [8192, 768] f32  (out[s] = weights[x[0, s]])

Strategy: data-parallel over the sequence. Each of the 8 cores gets the
full table (resident in its HBM) plus a 1024-token slice of ids, and
gathers its 1024 rows via SWDGE indirect DMA (one 3KB descriptor per
row) into SBUF, then stores each 128-row tile to its output slice.
Host concatenates the 8 slices. (A direct DRAM->DRAM gather would
halve the SDMA work, but that path is broken in hardware, as are
indirect DMAs touching fewer than 128 partitions.)

Raw Bass, no TileContext (its preamble + EVSEM drain tail cost ~16us
on a ~25us kernel). All buffers stay resident in SBUF so nothing
stalls on reuse. The ids load as one [128, 8] DMA, host-transposed so
gather t's offsets are column t (indirect offset APs must be [N, 1],
one index per partition — anything else dies on hardware). Stores
alternate between the two HWDGE engines (sync/scalar) with the last
tile's store split across both, and their semaphore is never waited
on: the exit-barrier Drain on each engine already waits for its
outstanding DMA data (verified in traces), so the kernel ends at drain
time instead of after another sem round-trip.
"""

import numpy as np

import concourse.bass as bass
import concourse.mybir as mybir
from concourse.bass_utils import run_bass_kernel_spmd

SEQ = 8192
VOCAB = 49408
DIM = 768
NCORES = 8
P = 128

TOK = SEQ // NCORES  # tokens per core
TILES = TOK // P  # gather tiles of P tokens

_cache = {}


def _build():
    """Per-core program: out[i, :] = weights[ids[i//64, i%64], :]."""
    import contextlib

    nc = bass.Bass()

    ids = nc.dram_tensor("ids", [P, TILES], mybir.dt.int32, kind="ExternalInput")
    weights = nc.dram_tensor(
        "weights", [VOCAB, DIM], mybir.dt.float32, kind="ExternalInput"
    )
    out = nc.dram_tensor("out", [TOK, DIM], mybir.dt.float32, kind="ExternalOutput")
    out_t = out.rearrange("(t p) d -> t p d", p=P)  # [TILES, P, DIM]

    with contextlib.ExitStack() as ctx:
        idx_sb = ctx.enter_context(nc.sbuf_tensor([P, TILES], mybir.dt.int32))
        g_sb = ctx.enter_context(nc.sbuf_tensor([P, TILES * DIM], mybir.dt.float32))
        idx_sem = ctx.enter_context(nc.semaphore("idx_sem"))
        store_sem = ctx.enter_context(nc.semaphore("store_sem"))
        # one sem per gather: DMA sem updates may not cross other DMAs'
        # waited values (race detector), so don't share a counter
        gather_sems = [
            ctx.enter_context(nc.semaphore(f"gather_sem{t}")) for t in range(TILES)
        ]
        block = ctx.enter_context(nc.Block())

        last = TILES - 1

        def store(eng, t, lo=0, hi=P):
            # store_sem is inc'd (every DMA must update a sem) but never
            # waited on: the engine's exit-barrier Drain already waits for
            # its outstanding DMA data, which is when the kernel may end
            eng.dma_start(
                out=out_t[t, lo:hi], in_=g_sb[lo:hi, t * DIM : (t + 1) * DIM]
            ).then_inc(store_sem, 16)

        @block.sync
        def _(sync: bass.BassEngine):
            sync.dma_start(out=idx_sb[:], in_=ids[:]).then_inc(idx_sem, 16)
            for t in range(0, TILES - 1, 2):
                sync.wait_ge(gather_sems[t], 16)
                store(sync, t)
            sync.wait_ge(gather_sems[last], 16)
            store(sync, last, 0, P // 2)

        @block.scalar
        def _(scalar: bass.BassEngine):
            for t in range(1, TILES - 1, 2):
                scalar.wait_ge(gather_sems[t], 16)
                store(scalar, t)
            scalar.wait_ge(gather_sems[last], 16)
            store(scalar, last, P // 2, P)

        @block.gpsimd
        def _(gpsimd: bass.BassEngine):
            gpsimd.wait_ge(idx_sem, 16)
            for t in range(TILES):
                gpsimd.indirect_dma_start(
                    out=g_sb[:, t * DIM : (t + 1) * DIM],
                    out_offset=None,
                    in_=weights[:],
                    in_offset=bass.IndirectOffsetOnAxis(
                        ap=idx_sb[:, t : t + 1], axis=0
                    ),
                ).then_inc(gather_sems[t], 16)

    return nc


def _get_nc():
    if "nc" not in _cache:
        _cache["nc"] = _build()
    return _cache["nc"]


def _run(x, weights, trace=False):
    ids = np.ascontiguousarray(np.asarray(x).reshape(-1).astype(np.int32))
    w = np.ascontiguousarray(np.asarray(weights, dtype=np.float32))
    assert ids.shape == (SEQ,) and w.shape == (VOCAB, DIM)

    nc = _get_nc()
    in_maps = [
        {
            # token t*P+p of this core's slice lands at idx_sb[p, t]
            "ids": np.ascontiguousarray(
                ids[c * TOK : (c + 1) * TOK].reshape(TILES, P).T
            ),
            "weights": w,
        }
        for c in range(NCORES)
    ]
    br = run_bass_kernel_spmd(nc, in_maps, list(range(NCORES)), trace=trace)
    out = np.concatenate([br.results[c]["out"] for c in range(NCORES)], axis=0)
    return out, br


def kernel(x, weights):
    out, _ = _run(x, weights)
    return out


# revision 24
# speedup vs baseline: 1.2519x; 1.0119x over previous
"""Embedding lookup kernel for Trainium2 (8 NeuronCores).

Problem: x [1, 8192] int token ids, weights [49408, 768] f32 table
         -> out [8192, 768] f32  (out[s] = weights[x[0, s]])

Strategy: data-parallel over the sequence. Each of the 8 cores gets the
full table (resident in its HBM) plus a 1024-token slice of ids, and
gathers its 1024 rows via SWDGE indirect DMA (one 3KB descriptor per
row) into SBUF, then stores each 128-row tile to its output slice.
Host concatenates the 8 slices. (A direct DRAM->DRAM gather would
halve the SDMA work, but that path is broken in hardware, as are
indirect DMAs touching fewer than 128 partitions.)

Raw Bass, no TileContext (its preamble + EVSEM drain tail cost ~16us
on a ~25us kernel). All buffers stay resident in SBUF so nothing
stalls on reuse. The ids load as one [128, 8] DMA, host-transposed so
gather t's offsets are column t (indirect offset APs must be [N, 1],
one index per partition — anything else dies on hardware). Stores
alternate between the two HWDGE engines (sync/scalar) with the last
tile's store split across both, and their semaphore is never waited
on: the exit-barrier Drain on each engine already waits for its
outstanding DMA data (verified in traces), so the kernel ends at drain
time instead of after another sem round-trip.
"""

import os

os.environ.setdefault("NEURON_RT_RESET_CORES", "1")

import numpy as np

import concourse.bass as bass
import concourse.mybir as mybir
from concourse.bass_utils import run_bass_kernel_spmd

SEQ = 8192
VOCAB = 49408
DIM = 768
NCORES = 8
P = 128

TOK = SEQ // NCORES  # tokens per core
TILES = TOK // P  # gather tiles of P tokens

_cache = {}


def _build():
    """Per-core program: out[i, :] = weights[ids[i//64, i%64], :]."""
    import contextlib

    nc = bass.Bass()

    ids = nc.dram_tensor("ids", [P, TILES], mybir.dt.int32, kind="ExternalInput")
    weights = nc.dram_tensor(
        "weights", [VOCAB, DIM], mybir.dt.float32, kind="ExternalInput"
    )
    out = nc.dram_tensor("out", [TOK, DIM], mybir.dt.float32, kind="ExternalOutput")
    out_t = out.rearrange("(t p) d -> t p d", p=P)  # [TILES, P, DIM]

    with contextlib.ExitStack() as ctx:
        idx_sb = ctx.enter_context(nc.sbuf_tensor([P, TILES], mybir.dt.int32))
        g_sb = ctx.enter_context(nc.sbuf_tensor([P, TILES * DIM], mybir.dt.float32))
        idx_sem = ctx.enter_context(nc.semaphore("idx_sem"))
        store_sem = ctx.enter_context(nc.semaphore("store_sem"))
        # one sem per gather: DMA sem updates may not cross other DMAs'
        # waited values (race detector), so don't share a counter
        gather_sems = [
            ctx.enter_context(nc.semaphore(f"gather_sem{t}")) for t in range(TILES)
        ]
        block = ctx.enter_context(nc.Block())

        last = TILES - 1

        def store(eng, t, lo=0, hi=P):
            # store_sem is inc'd (every DMA must update a sem) but never
            # waited on: the engine's exit-barrier Drain already waits for
            # its outstanding DMA data, which is when the kernel may end
            eng.dma_start(
                out=out_t[t, lo:hi], in_=g_sb[lo:hi, t * DIM : (t + 1) * DIM]
            ).then_inc(store_sem, 16)

        @block.sync
        def _(sync: bass.BassEngine):
            sync.dma_start(out=idx_sb[:], in_=ids[:]).then_inc(idx_sem, 16)
            for t in range(0, TILES - 1, 2):
                sync.wait_ge(gather_sems[t], 16)
                store(sync, t)
            sync.wait_ge(gather_sems[last], 16)
            store(sync, last, 0, P // 2)

        @block.scalar
        def _(scalar: bass.BassEngine):
            for t in range(1, TILES - 1, 2):
                scalar.wait_ge(gather_sems[t], 16)
                store(scalar, t)
            scalar.wait_ge(gather_sems[last], 16)
            store(scalar, last, P // 2, P)

        @block.gpsimd
        def _(gpsimd: bass.BassEngine):
            gpsimd.wait_ge(idx_sem, 16)
            for t in range(TILES):
                gpsimd.indirect_dma_start(
                    out=g_sb[:, t * DIM : (t + 1) * DIM],
                    out_offset=None,
                    in_=weights[:],
                    in_offset=bass.IndirectOffsetOnAxis(
                        ap=idx_sb[:, t : t + 1], axis=0
                    ),
                ).then_inc(gather_sems[t], 16)

    return nc


def _get_nc():
    if "nc" not in _cache:
        _cache["nc"] = _build()
    return _cache["nc"]


def _run(x, weights, trace=False):
    ids = np.ascontiguousarray(np.asarray(x).reshape(-1).astype(np.int32))
    w = np.ascontiguousarray(np.asarray(weights, dtype=np.float32))
    assert ids.shape == (SEQ,) and w.shape == (VOCAB, DIM)

    nc = _get_nc()
    in_maps = [
        {
            # token t*P+p of this core's slice lands at idx_sb[p, t]
            "ids": np.ascontiguousarray(
                ids[c * TOK : (c + 1) * TOK].reshape(TILES, P).T
            ),
            "weights": w,
        }
        for c in range(NCORES)
    ]
    br = run_bass_kernel_spmd(nc, in_maps, list(range(NCORES)), trace=trace)
    out = np.concatenate([br.results[c]["out"] for c in range(NCORES)], axis=0)
    return out, br


def kernel(x, weights):
    out, _ = _run(x, weights)
    return out
